# revision 1
# baseline (speedup 1.0000x reference)
"""Trainium2 kernel for nn_MultiHeadClassifier.

Math: out[i] = W[task_labels[i]] @ x[i] + b[task_labels[i]]
  x [262144, 1024] f32, task_labels [262144] int, W [8, 32, 1024], b [8, 32]

Strategy (8 NeuronCores, data-parallel over batch):
  - Each core gets 32768 rows. x is staged in HBM transposed
    ([8, 128, 32768]: k-tile, d-within-tile, row) so the PE can contract
    over d (partition dim) directly.
  - All T=8 heads are computed at once per 128-row tile: 8 float32r
    matmuls (full PE rate, ~1.5e-4 rel err) accumulate y = x @ Wflat.T
    ([128 rows, 256]) in PSUM, plus a K=1 bf16 matmul adding the bias.
  - Head selection (the MoE routing) happens on-device on the DVE:
    one-hot mask [128, 8] broadcast-multiplied into y viewed [128, 8, 32],
    then a strided reduce over the 8 task slots -> out tile [128, 32].
  - Output is written in [128, 256, 32] (partition-major) layout with
    fully contiguous per-partition DMA runs; host reshapes back.
"""

import sys

sys.path.insert(0, "/opt/trn_rl_repo")

import numpy as np
import ml_dtypes

import concourse.bass as bass
import concourse.tile as tile
from concourse import bacc, mybir
from concourse import bass_utils

B, D, C, T = 262144, 1024, 32, 8
NCORES = 8
N = B // NCORES  # 32768 rows per core
P = 128
KO = D // P  # 8 contraction tiles
TC = T * C  # 256 = all-heads output width
SB = 1024  # rows per superblock (one x DMA)
NT = N // P  # 256 row-tiles per core
SBT = SB // P  # row-tiles per superblock
NSB = N // SB  # superblocks per core

# set by test harness to collect a profile; harness-invoked kernel() keeps it off
TRACE = False
LAST_RESULTS = None


def _build():
    f32 = mybir.dt.float32
    f32r = mybir.dt.float32r
    bf16 = mybir.dt.bfloat16

    nc = bacc.Bacc("TRN2", debug=False, num_devices=NCORES)
    # xt[sb, ki, ko, r]: one superblock is a contiguous 2 MB region with
    # 16 KB contiguous per partition -> near-peak DMA efficiency.
    xt_d = nc.dram_tensor("xt", [NSB, P, KO, SB], f32r, kind="ExternalInput")
    wft_d = nc.dram_tensor("wft", [KO, P, TC], f32r, kind="ExternalInput")
    mask_d = nc.dram_tensor("mask8", [P, NT, T], f32, kind="ExternalInput")
    # bpack[0, :P] = ones, bpack[0, P:] = b.reshape(256) twice (bf16)
    bpack_d = nc.dram_tensor("bpack", [1, P + 2 * TC], bf16, kind="ExternalInput")
    out_d = nc.dram_tensor("out", [P, NT, C], f32, kind="ExternalOutput")

    with tile.TileContext(nc) as tc:
        with (
            tc.tile_pool(name="consts", bufs=1) as consts,
            tc.tile_pool(name="xpool", bufs=5) as xpool,
            tc.tile_pool(name="work", bufs=8) as work,
            tc.tile_pool(name="opool", bufs=3) as opool,
            tc.tile_pool(name="psum", bufs=8, space="PSUM") as psum,
        ):
            # first x superblock in flight before the consts
            xts0 = xpool.tile([P, KO, SB], f32r, tag="xts")
            nc.sync.dma_start(xts0[:], xt_d[0])

            # consts on the ACT ring: the SP ring stays a pure x stream
            wft = consts.tile([P, KO, TC], f32r)
            nc.scalar.dma_start(wft[:], wft_d[:].rearrange("ko ki n -> ki ko n"))
            mask8 = consts.tile([P, NT, T], f32)
            nc.scalar.dma_start(mask8[:], mask_d[:])
            bpack = consts.tile([1, P + 2 * TC], bf16)
            nc.scalar.dma_start(bpack[:], bpack_d[:])
            ones1 = bpack[:, :P]
            bexp2 = bpack[:, P:]  # [1, 512] = b flat, twice

            # Engine warmups: with the 1-sync-wait-per-instruction ISA
            # limit, give each engine one instruction that observes the
            # const DMA lanes, so steady-state instructions carry at most
            # one wait each.
            scratch = psum.tile([P, TC], mybir.dt.float32, tag="y")
            w0 = wft[:, 0, :1].bitcast(bf16)  # [P, 2] garbage bf16 view
            nc.tensor.matmul(scratch[:2, :2], w0, w0, start=True, stop=True)
            dve_scr = work.tile([P, T], f32, tag="dve_scr")
            nc.vector.tensor_copy(dve_scr[:], mask8[:, 0, :])

            for sb in range(NSB):
                if sb == 0:
                    xts = xts0
                else:
                    xts = xpool.tile([P, KO, SB], f32r, tag="xts")
                    nc.sync.dma_start(xts[:], xt_d[sb])
                out_sb = opool.tile([P, SBT, C], f32, tag="out_sb")
                for st in range(SBT):
                    ro = sb * SBT + st
                    y = psum.tile([P, TC], mybir.dt.float32, tag="y")
                    # bias first: absorbs the psum-slot WAR wait; single
                    # const producer (bpack DMA).
                    nc.tensor.matmul(
                        y[:], ones1, bexp2[:, :TC], start=True, stop=False
                    )
                    for ko in range(KO):
                        nc.tensor.matmul(
                            y[:],
                            xts[:, ko, st * P : (st + 1) * P],
                            wft[:, ko, :],
                            start=False,
                            stop=(ko == KO - 1),
                        )
                    # tmp[p, t, c] = y[p, t*C+c] * mask8[p, ro, t]
                    tmp = work.tile([P, TC], f32, tag="tmp")
                    nc.vector.tensor_tensor(
                        tmp[:].rearrange("p (t c) -> p t c", t=T),
                        y[:].rearrange("p (t c) -> p t c", t=T),
                        mask8[:, ro, :, None].to_broadcast((P, T, C)),
                        mybir.AluOpType.mult,
                    )
                    # out[p, c] = sum_t tmp[p, t, c]
                    nc.vector.tensor_reduce(
                        out_sb[:, st, :],
                        tmp[:].rearrange("p (t c) -> p c t", t=T),
                        axis=mybir.AxisListType.X,
                        op=mybir.AluOpType.add,
                    )
                # out on the ACT HWDGE ring so it never delays xts loads
                # queued on the SP ring
                nc.scalar.dma_start(
                    out_d[:, sb * SBT : (sb + 1) * SBT, :], out_sb[:]
                )
    nc.compile()
    return nc


_NC = None


def _get_nc():
    global _NC
    if _NC is None:
        _NC = _build()
    return _NC


def kernel(x, task_labels, W, b):
    global LAST_RESULTS
    x = np.asarray(x)
    if x.dtype != np.float32:
        x = x.astype(np.float32)
    labels = np.asarray(task_labels).astype(np.int32)
    W = np.asarray(W)
    if W.dtype != np.float32:
        W = W.astype(np.float32)
    b = np.asarray(b)
    if b.dtype != np.float32:
        b = b.astype(np.float32)

    wft = np.ascontiguousarray(W.reshape(TC, D).T).reshape(KO, P, TC)
    bpack = (
        np.concatenate(
            [np.ones(P, np.float32), b.reshape(TC), b.reshape(TC)]
        )
        .reshape(1, P + 2 * TC)
        .astype(ml_dtypes.bfloat16)
    )
    tids = np.arange(T, dtype=np.int32)[None, None, :]

    in_maps = []
    for c in range(NCORES):
        xs = x[c * N : (c + 1) * N]
        ls = labels[c * N : (c + 1) * N]
        # xt[sb, ki, ko, r] = xs[sb*SB + r, ko*P + ki]
        xt = np.ascontiguousarray(
            xs.reshape(NSB, SB, KO, P).transpose(0, 3, 2, 1)
        )
        lab2 = ls.reshape(NT, P).T  # [P, NT]
        mask8 = (lab2[:, :, None] == tids).astype(np.float32)
        in_maps.append(
            {"xt": xt, "wft": wft, "mask8": mask8, "bpack": bpack}
        )

    nc = _get_nc()
    res = bass_utils.run_bass_kernel_spmd(
        nc, in_maps, core_ids=list(range(NCORES)), trace=TRACE
    )
    LAST_RESULTS = res
    outs = [
        r["out"].transpose(1, 0, 2).reshape(N, C) for r in res.results
    ]
    return np.concatenate(outs, axis=0)



# revision 2
# speedup vs baseline: 2.0483x; 2.0483x over previous
"""Trainium2 kernel for nn_MultiHeadClassifier.

Math: out[i] = W[task_labels[i]] @ x[i] + b[task_labels[i]]
  x [262144, 1024] f32, task_labels [262144] int, W [8, 32, 1024], b [8, 32]

Strategy (8 NeuronCores, routed data-parallel over batch):
  - Host routes rows by task: for each task t, its rows are split evenly
    across the 8 cores and padded up to whole 128-row tiles, so every
    tile on device is single-task. The per-task tile counts A[t] (same
    on every core by construction) parameterize the compiled schedule;
    compilation is cached keyed on A.
  - x is staged in HBM as bf16, transposed ([sb, 128, 8, 1024]: k-tile,
    d-within-tile, row) so the PE contracts over d (partition dim)
    directly. bf16 halves the dominant HBM traffic (this problem is
    memory-bound); PSUM accumulation stays f32, rel err ~3e-3.
  - Per 128-row tile only the tile's own head is computed: 8 bf16
    matmuls with x as the stationary operand and W[t] k-slices moving
    (output free size 32), plus a K=1 matmul adding the bias. ~9x less
    PE work than computing all 8 heads.
  - Output is written in [128, NTP, 32] (partition-major) layout with
    contiguous per-partition DMA runs; host scatters rows back through
    the routing permutation.
"""

import sys

sys.path.insert(0, "/opt/trn_rl_repo")

import numpy as np
import ml_dtypes

import concourse.bass as bass
import concourse.tile as tile
from concourse import bacc, mybir
from concourse import bass_utils

B, D, C, T = 262144, 1024, 32, 8
NCORES = 8
N = B // NCORES  # 32768 rows per core (pre-routing)
P = 128
KO = D // P  # 8 contraction tiles
SB = 1024  # rows per superblock (one x DMA)
SBT = SB // P  # row-tiles per superblock

# set by test harness to collect a profile; harness-invoked kernel() keeps it off
TRACE = False
LAST_RESULTS = None


def _schedule(counts):
    """Per-task tiles-per-core A[t] and the flat per-tile task schedule."""
    A = [int(-(-int(c) // (NCORES * P))) for c in counts]  # ceil
    ntp = sum(A)
    A[T - 1] += (-ntp) % SBT  # pad tile count to whole superblocks
    sched = []
    for t in range(T):
        sched.extend([t] * A[t])
    return tuple(A), sched


def _build(a_key):
    f32 = mybir.dt.float32
    bf16 = mybir.dt.bfloat16

    A = list(a_key)
    ntp = sum(A)
    nsb = ntp // SBT
    sched = []
    for t in range(T):
        sched.extend([t] * A[t])

    nc = bacc.Bacc("TRN2", debug=False, num_devices=NCORES)
    # xt[sb, ki, ko, r]: one superblock is a contiguous 2 MB region with
    # 16 KB contiguous per partition -> near-peak DMA efficiency.
    xt_d = nc.dram_tensor("xt", [nsb, P, KO, SB], bf16, kind="ExternalInput")
    # wsb[ki, t, ko, c] = W[t, c, ko*128+ki]
    wsb_d = nc.dram_tensor("wsb", [P, T, KO, C], bf16, kind="ExternalInput")
    # bpack[0, :P] = ones, bpack[0, P:] = b.reshape(256) (bf16)
    bpack_d = nc.dram_tensor("bpack", [1, P + T * C], bf16, kind="ExternalInput")
    out_d = nc.dram_tensor("out", [P, ntp, C], f32, kind="ExternalOutput")

    with tile.TileContext(nc) as tc:
        with (
            tc.tile_pool(name="consts", bufs=1) as consts,
            tc.tile_pool(name="xpool", bufs=6) as xpool,
            tc.tile_pool(name="opool", bufs=3) as opool,
            tc.tile_pool(name="psum", bufs=4, space="PSUM") as psum,
        ):
            # first x superblock in flight before the consts
            xts0 = xpool.tile([P, KO, SB], bf16, tag="xts")
            nc.sync.dma_start(xts0[:], xt_d[0])

            # consts on the ACT ring: the SP ring stays a pure x stream
            wsb = consts.tile([P, T, KO, C], bf16)
            nc.scalar.dma_start(wsb[:], wsb_d[:])
            bpack = consts.tile([1, P + T * C], bf16)
            nc.scalar.dma_start(bpack[:], bpack_d[:])
            ones1 = bpack[:, :P]  # [1, 128]

            # Engine warmups: with the 1-sync-wait-per-instruction ISA
            # limit, give the PE one instruction per const DMA lane so
            # steady-state instructions carry at most one wait each.
            scratch = psum.tile([P, SBT, C], f32, tag="y")
            w0 = wsb[:, 0, 0, :2]  # [128, 2]
            nc.tensor.matmul(scratch[:2, 0, :2], w0, w0, start=True, stop=True)
            nc.tensor.matmul(
                scratch[:2, 0, :2],
                bpack[:, :2],
                bpack[:, :2],
                start=True,
                stop=True,
            )

            for sb in range(nsb):
                if sb == 0:
                    xts = xts0
                else:
                    xts = xpool.tile([P, KO, SB], bf16, tag="xts")
                    nc.sync.dma_start(xts[:], xt_d[sb])
                out_sb = opool.tile([P, SBT, C], f32, tag="out_sb")
                y = psum.tile([P, SBT, C], f32, tag="y")
                for st in range(SBT):
                    t = sched[sb * SBT + st]
                    # bias first: absorbs the psum-slot WAR wait; single
                    # const producer (bpack DMA).
                    nc.tensor.matmul(
                        y[:, st, :],
                        ones1,
                        bpack[:, P + t * C : P + (t + 1) * C],
                        start=True,
                        stop=False,
                    )
                    for ko in range(KO):
                        nc.tensor.matmul(
                            y[:, st, :],
                            xts[:, ko, st * P : (st + 1) * P],
                            wsb[:, t, ko, :],
                            start=False,
                            stop=(ko == KO - 1),
                        )
                nc.vector.tensor_copy(out_sb[:], y[:])
                # out on the ACT HWDGE ring so it never delays xts loads
                # queued on the SP ring
                nc.scalar.dma_start(
                    out_d[:, sb * SBT : (sb + 1) * SBT, :], out_sb[:]
                )
    nc.compile()
    return nc


_NC_CACHE = {}


def _get_nc(a_key):
    if a_key not in _NC_CACHE:
        _NC_CACHE[a_key] = _build(a_key)
    return _NC_CACHE[a_key]


def prebuild(task_labels):
    """Optional: compile ahead of kernel() for these labels."""
    labels = np.asarray(task_labels).astype(np.int32)
    counts = np.bincount(labels, minlength=T)
    a_key, _ = _schedule(counts)
    return _get_nc(a_key)


def kernel(x, task_labels, W, b):
    global LAST_RESULTS
    x = np.asarray(x)
    if x.dtype != np.float32:
        x = x.astype(np.float32)
    labels = np.asarray(task_labels).astype(np.int32)
    W = np.asarray(W).astype(np.float32)
    b = np.asarray(b).astype(np.float32)

    counts = np.bincount(labels, minlength=T)
    a_key, sched = _schedule(counts)
    A = list(a_key)
    ntp = sum(A)
    nsb = ntp // SBT
    npad = ntp * P  # padded rows per core

    # Per-task row routing: task t's rows split evenly across cores.
    by_task = [np.flatnonzero(labels == t) for t in range(T)]
    chunk = [-(-len(ix) // NCORES) for ix in by_task]

    wsbh = np.ascontiguousarray(
        W.reshape(T, C, KO, P).transpose(3, 0, 2, 1)
    ).astype(ml_dtypes.bfloat16)
    bpack = (
        np.concatenate([np.ones(P, np.float32), b.reshape(T * C)])
        .reshape(1, P + T * C)
        .astype(ml_dtypes.bfloat16)
    )

    in_maps = []
    idx_pads = []
    real_masks = []
    xbf = x.astype(ml_dtypes.bfloat16)
    for c in range(NCORES):
        idx_pad = np.zeros(npad, np.int64)
        real = np.zeros(npad, bool)
        ofs = 0
        for t in range(T):
            cap = A[t] * P
            seg = by_task[t][c * chunk[t] : (c + 1) * chunk[t]]
            idx_pad[ofs : ofs + len(seg)] = seg
            if len(seg):
                idx_pad[ofs + len(seg) : ofs + cap] = seg[0]
            real[ofs : ofs + len(seg)] = True
            ofs += cap
        xs = xbf[idx_pad]
        # xt[sb, ki, ko, r] = xs[sb*SB + r, ko*P + ki]
        xt = np.ascontiguousarray(
            xs.reshape(nsb, SB, KO, P).transpose(0, 3, 2, 1)
        )
        idx_pads.append(idx_pad)
        real_masks.append(real)
        in_maps.append({"xt": xt, "wsb": wsbh, "bpack": bpack})

    nc = _get_nc(a_key)
    res = bass_utils.run_bass_kernel_spmd(
        nc, in_maps, core_ids=list(range(NCORES)), trace=TRACE
    )
    LAST_RESULTS = res
    out = np.empty((B, C), np.float32)
    for c in range(NCORES):
        rows = (
            res.results[c]["out"].transpose(1, 0, 2).reshape(npad, C)
        )
        sel = real_masks[c]
        out[idx_pads[c][sel]] = rows[sel]
    return out
